# revision 10
# baseline (speedup 1.0000x reference)
"""Cross-modal channel attention (CrossModelAtt) Bass/Tile kernel for TRN2.

Reference computation per batch b (C=512, N=HW=4096):
    q  = img[b]            # [C, N]
    kv = text[b]           # [C, N]
    S  = q @ kv.T          # [C, C]
    P  = softmax(S, -1)
    out[b] = gamma * (P @ kv) + img[b]

Sharding: pure data-parallel over batch; 16 batches / 8 cores = 2 per core.

Device pipeline (per batch), engineered against the TRN2 cost model:
  - host pre-packs inputs: img (residual) in bf16 p-major layout; q
    pre-transposed to [n,c] fp8e4m3 (mm1 stationary); kv in fp8e4m3
    p-major layout (mm2 moving); kv pre-transposed to [n,d] fp8e4m3
    (mm1 moving).  All layout/dtype prep is host-side preprocessing.
  - both matmuls run fp8 DoubleRow (K=256 per instruction, 0.5 cyc/row)
  - mm1 ci-outer on a 2-deep S PSUM ring; softmax: DVE max(negate) ->
    ACT exp(bias=-max, accum_out=rowsum) -> DVE reciprocal -> P scaled
    by gamma/rowsum on DVE (folds gamma)
  - pT via bf16 PE transposes, DVE evac converts to fp8
  - mm2 accumulates [128,512] info tiles on a 5-deep PSUM ring; the
    residual alternates DVE tensor_add with PE identity-add + ACT copy
    so two engines drain the ring in parallel (GPSIMD cannot read PSUM)
  - bf16 out tiles stored from the SP queue; host converts bf16 -> fp32
"""

import numpy as np

B, C, H, W = 16, 512, 64, 64
N = H * W                 # 4096
N_CORES = 8
BPC = B // N_CORES        # batches per core
CP = C // 128             # 4 c-chunks
NJ = N // 128             # 32 n-chunks
NT = NJ // 2              # 16 DoubleRow kpairs for mm1

_nc_cache = None


def _build_nc():
    import concourse.mybir as mybir
    from concourse import bacc
    from concourse.tile import TileContext
    from concourse.masks import make_identity

    F32 = mybir.dt.float32
    BF16 = mybir.dt.bfloat16
    FP8 = mybir.dt.float8e4
    AX = mybir.AxisListType.X
    DR = mybir.MatmulPerfMode.DoubleRow

    nc = bacc.Bacc("TRN2", target_bir_lowering=False, debug=False,
                   num_devices=N_CORES)

    # host-prepacked layouts (p = partition):
    #   img: [BPC*128, CP*N]  bf16, img[b*128+p, cj*N+n] = img[b, cj*128+p, n]
    #   qt : [BPC*128, NJ*C]  fp8,  qt[b*128+p, nj*C+c] = img[b, c, nj*128+p]
    #   kv : [BPC*128, CP*N]  fp8,  kv[b*128+p, dj*N+n] = txt[b, dj*128+p, n]
    #   kvt: [BPC*128, NJ*C]  fp8,  kvt[b*128+p, nj*C+d] = txt[b, d, nj*128+p]
    img_d = nc.dram_tensor("img", [BPC * 128, CP * N], BF16,
                           kind="ExternalInput")
    qt_d = nc.dram_tensor("qt", [BPC * 128, NJ * C], FP8,
                          kind="ExternalInput")
    kv_d = nc.dram_tensor("kv", [BPC * 128, CP * N], FP8,
                          kind="ExternalInput")
    kvt_d = nc.dram_tensor("kvt", [BPC * 128, NJ * C], FP8,
                           kind="ExternalInput")
    gam_d = nc.dram_tensor("gam", [1], F32, kind="ExternalInput")
    out_d = nc.dram_tensor("out", [BPC * C, N], BF16, kind="ExternalOutput")

    img_f = img_d.ap()
    qt_f = qt_d.ap()
    kv_f = kv_d.ap()
    kvt_f = kvt_d.ap()
    out_f = out_d.ap()

    with TileContext(nc) as tc:
        with (
            tc.tile_pool(name="const", bufs=1) as const_pool,
            tc.tile_pool(name="kv", bufs=2) as kv_pool,        # [128,16K] fp8
            tc.tile_pool(name="kvt", bufs=2) as kvt_pool,      # [128,16K] fp8
            tc.tile_pool(name="qt", bufs=2) as qt_pool,        # [128,16K] fp8
            tc.tile_pool(name="img", bufs=8) as img_pool,      # [128,4096] bf16
            tc.tile_pool(name="p", bufs=8) as p_pool,          # [128,512] bf16
            tc.tile_pool(name="pt", bufs=2) as pt_pool,        # [128,2048] fp8
            tc.tile_pool(name="ot", bufs=5) as out_pool,       # [128,2048] bf16
            tc.tile_pool(name="stat", bufs=16) as stat_pool,   # [128,1] f32
            tc.tile_pool(name="sps", bufs=2, space="PSUM") as s_pool,
            tc.tile_pool(name="pts", bufs=1, space="PSUM") as pt_ps_pool,
            tc.tile_pool(name="ips", bufs=5, space="PSUM") as info_pool,
        ):
            ident_bf = const_pool.tile([128, 128], BF16, tag="identbf")
            make_identity(nc, ident_bf[:])
            g1 = const_pool.tile([128, 1], F32, tag="g1")
            nc.sync.dma_start(g1[0:1, 0:1],
                              gam_d.ap().rearrange("(a b) -> a b", a=1))
            g_b = const_pool.tile([128, 1], F32, tag="gb")
            nc.gpsimd.partition_broadcast(g_b[:], g1[0:1, 0:1])

            # ---------------- prologue: all loads on SP ----------------
            kv_sb, kvt_sb, qt_sb, img_t = [], [], [], []
            for b in range(BPC):
                prows = slice(b * 128, (b + 1) * 128)
                kvtt = kvt_pool.tile([128, NJ * C], FP8, tag="kvt",
                                     name=f"kvt_{b}")
                for h in range(2):
                    nc.sync.dma_start(kvtt[:, h * 8192:(h + 1) * 8192],
                                      kvt_f[prows, h * 8192:(h + 1) * 8192])
                qtt = qt_pool.tile([128, NJ * C], FP8, tag="qt",
                                   name=f"qt_{b}")
                for h in range(2):
                    nc.sync.dma_start(qtt[:, h * 8192:(h + 1) * 8192],
                                      qt_f[prows, h * 8192:(h + 1) * 8192])
                imgs = [img_pool.tile([128, N], BF16, tag="img",
                                      name=f"img_{b}_{ci}")
                        for ci in range(CP)]
                kvt_ = kv_pool.tile([128, CP * N], FP8, tag="kv",
                                    name=f"kv_{b}")
                # interleave kv halves with the first img chunks so mm2 and
                # the residual unblock at similar times
                nc.sync.dma_start(kvt_[:, 0:2 * N], kv_f[prows, 0:2 * N])
                nc.sync.dma_start(imgs[0][:], img_f[prows, 0:N])
                nc.sync.dma_start(kvt_[:, 2 * N:4 * N],
                                  kv_f[prows, 2 * N:4 * N])
                for ci in range(1, CP):
                    nc.sync.dma_start(imgs[ci][:],
                                      img_f[prows, ci * N:(ci + 1) * N])
                kv_sb.append(kvt_)
                kvt_sb.append(kvtt)
                qt_sb.append(qtt)
                img_t.append(imgs)

            kv3 = [t[:].rearrange("p (d n) -> p d n", n=N) for t in kv_sb]
            qt3 = [t[:].rearrange("p (j c) -> p j c", c=C) for t in qt_sb]
            kvt3 = [t[:].rearrange("p (j d) -> p j d", d=C) for t in kvt_sb]

            for b in range(BPC):
                # ---------- mm1, ci-outer on a 2-deep S ring ----------
                psc = []
                for ci in range(CP):
                    s_ps = s_pool.tile([128, 512], F32, tag="s",
                                       name=f"s_{b}_{ci}")
                    for t in range(NT):
                        nc.tensor.matmul(
                            s_ps[:],
                            qt3[b][:, 2 * t:2 * t + 2,
                                   ci * 128:(ci + 1) * 128],
                            kvt3[b][:, 2 * t:2 * t + 2, :],
                            start=(t == 0), stop=(t == NT - 1),
                            perf_mode=DR)

                    # ---------- softmax (gamma/rowsum folded into P) ------
                    nm = stat_pool.tile([128, 1], F32, tag="nm")
                    nc.vector.reduce_max(nm[:], s_ps[:], axis=AX,
                                         negate=True)
                    praw = p_pool.tile([128, 512], BF16, tag="praw")
                    rs = stat_pool.tile([128, 1], F32, tag="rs")
                    nc.scalar.activation(praw[:], s_ps[:],
                                         mybir.ActivationFunctionType.Exp,
                                         bias=nm[:], scale=1.0,
                                         accum_out=rs[:])
                    rr = stat_pool.tile([128, 1], F32, tag="rr")
                    nc.vector.reciprocal(rr[:], rs[:])
                    rg = stat_pool.tile([128, 1], F32, tag="rg")
                    nc.vector.tensor_mul(rg[:], rr[:], g_b[:])
                    ps = p_pool.tile([128, 512], BF16, tag="ps")
                    nc.vector.tensor_scalar_mul(ps[:], praw[:], rg[:])
                    psc.append(ps)

                # ---------- pT: bf16 PE transposes, DVE evac -> fp8 -------
                pt_sb = pt_pool.tile([128, CP * C], FP8, tag="pt",
                                     name=f"pt_{b}")
                pt3 = pt_sb[:].rearrange("p (d c) -> p d c", c=C)
                for hh in range(2):
                    ptp = pt_ps_pool.tile([128, 1024], BF16, tag="ptp")
                    pv = ptp[:].rearrange("p (l c) -> p l c", c=128)
                    for cl in range(2):
                        ci = hh * 2 + cl
                        for dj in range(CP):
                            nc.tensor.transpose(
                                pv[:, cl * 4 + dj, :],
                                psc[ci][:, dj * 128:(dj + 1) * 128],
                                ident_bf[:])
                    for cl in range(2):
                        ci = hh * 2 + cl
                        nc.vector.tensor_copy(
                            pt3[:, :, ci * 128:(ci + 1) * 128],
                            pv[:, cl * 4:(cl + 1) * 4, :])

                # ---------- mm2 + residual + store ----------
                res_idx = 0
                for ci in range(CP):
                    orow = slice(b * C + ci * 128, b * C + (ci + 1) * 128)
                    for h in range(2):
                        ot = out_pool.tile([128, 2048], BF16, tag="ot",
                                           name=f"ot_{b}_{ci}_{h}")
                        for u in range(4):
                            nb = h * 4 + u
                            cols = slice(nb * 512, (nb + 1) * 512)
                            ip = info_pool.tile([128, 512], F32, tag="i")
                            for t in range(2):
                                nc.tensor.matmul(
                                    ip[:],
                                    pt3[:, 2 * t:2 * t + 2,
                                        ci * 128:(ci + 1) * 128],
                                    kv3[b][:, 2 * t:2 * t + 2,
                                           nb * 512:(nb + 1) * 512],
                                    start=(t == 0), stop=(t == 1),
                                    perf_mode=DR)
                            oc = ot[:, u * 512:(u + 1) * 512]
                            if res_idx % 2 == 0:
                                nc.vector.tensor_add(oc, ip[:],
                                                     img_t[b][ci][:, cols])
                            else:
                                # PE adds the bf16 residual into the open
                                # accumulator; ACT evacuates
                                nc.tensor.matmul(
                                    ip[:], ident_bf[:],
                                    img_t[b][ci][:, cols],
                                    start=False, stop=True,
                                    skip_group_check=True)
                                nc.scalar.copy(oc, ip[:])
                            res_idx += 1
                        nc.sync.dma_start(
                            out_f[orow, h * 2048:(h + 1) * 2048], ot[:])

    nc.compile()
    return nc


def _get_nc():
    global _nc_cache
    if _nc_cache is None:
        _nc_cache = _build_nc()
    return _nc_cache


def kernel(img_feat, text_feat, gamma):
    import ml_dtypes
    from concourse.bass_utils import run_bass_kernel_spmd

    nc = _get_nc()
    BF = ml_dtypes.bfloat16
    F8 = ml_dtypes.float8_e4m3

    img = np.asarray(img_feat, dtype=np.float32).reshape(B, C, N)
    txt = np.asarray(text_feat, dtype=np.float32).reshape(B, C, N)
    g = np.ascontiguousarray(np.asarray(gamma), dtype=np.float32).reshape(1)

    def pmajor(x):      # [B, C, N] -> [B*128, CP*N], partition-major rows
        return x.reshape(B, CP, 128, N).transpose(0, 2, 1, 3).reshape(
            B * 128, CP * N)

    def tmajor(x):      # [B, C, N] -> [B*128, NJ*C], [n, c] partition-major
        return x.transpose(0, 2, 1).reshape(B, NJ, 128, C).transpose(
            0, 2, 1, 3).reshape(B * 128, NJ * C)

    img_bf = np.ascontiguousarray(pmajor(img).astype(BF))
    qt_f8 = np.ascontiguousarray(tmajor(img).astype(F8))
    kv_f8 = np.ascontiguousarray(pmajor(txt).astype(F8))
    kvt_f8 = np.ascontiguousarray(tmajor(txt).astype(F8))

    R = BPC * 128
    in_maps = [
        {
            "img": img_bf[i * R:(i + 1) * R],
            "qt": qt_f8[i * R:(i + 1) * R],
            "kv": kv_f8[i * R:(i + 1) * R],
            "kvt": kvt_f8[i * R:(i + 1) * R],
            "gam": g,
        }
        for i in range(N_CORES)
    ]
    res = run_bass_kernel_spmd(nc, in_maps, core_ids=list(range(N_CORES)))
    out = np.concatenate(
        [np.asarray(res.results[i]["out"]) for i in range(N_CORES)], axis=0)
    return out.astype(np.float32).reshape(B, C, H, W)


# revision 15
# speedup vs baseline: 1.0218x; 1.0218x over previous
"""Cross-modal channel attention (CrossModelAtt) Bass/Tile kernel for TRN2.

Reference computation per batch b (C=512, N=HW=4096):
    q  = img[b]            # [C, N]
    kv = text[b]           # [C, N]
    S  = q @ kv.T          # [C, C]
    P  = softmax(S, -1)
    out[b] = gamma * (P @ kv) + img[b]

Sharding: pure data-parallel over batch; 16 batches / 8 cores = 2 per core.

Device pipeline (per batch), engineered against the TRN2 cost model:
  - host pre-packs inputs: img (residual) in bf16 p-major layout; q
    pre-transposed to [n,c] fp8e4m3 (mm1 stationary); kv in fp8e4m3
    p-major layout (mm2 moving); kv pre-transposed to [n,d] fp8e4m3
    (mm1 moving).  All layout/dtype prep is host-side preprocessing.
  - both matmuls run fp8 DoubleRow (K=256 per instruction, 0.5 cyc/row)
  - mm1 ci-outer on a 2-deep S PSUM ring; softmax: DVE max(negate) ->
    ACT exp(bias=-max, accum_out=rowsum) -> DVE reciprocal -> P scaled
    by gamma/rowsum on DVE (folds gamma)
  - pT via bf16 PE transposes, DVE evac converts to fp8
  - mm2 accumulates [128,512] info tiles on a 5-deep PSUM ring; the
    residual alternates DVE tensor_add with PE identity-add + ACT copy
    so two engines drain the ring in parallel (GPSIMD cannot read PSUM)
  - bf16 out tiles stored from the SP queue; host converts bf16 -> fp32
"""

import numpy as np

B, C, H, W = 16, 512, 64, 64
N = H * W                 # 4096
N_CORES = 8
BPC = B // N_CORES        # batches per core
CP = C // 128             # 4 c-chunks
NJ = N // 128             # 32 n-chunks
NT = NJ // 2              # 16 DoubleRow kpairs for mm1

_nc_cache = None


def _build_nc():
    import concourse.mybir as mybir
    from concourse import bacc
    from concourse.tile import TileContext
    from concourse.masks import make_identity

    F32 = mybir.dt.float32
    BF16 = mybir.dt.bfloat16
    FP8 = mybir.dt.float8e4
    AX = mybir.AxisListType.X
    DR = mybir.MatmulPerfMode.DoubleRow

    nc = bacc.Bacc("TRN2", target_bir_lowering=False, debug=False,
                   num_devices=N_CORES)

    # host-prepacked layouts (p = partition):
    #   img: [BPC*128, CP*N]  bf16, img[b*128+p, cj*N+n] = img[b, cj*128+p, n]
    #   qt : [BPC*128, NJ*C]  fp8,  qt[b*128+p, nj*C+c] = img[b, c, nj*128+p]
    #   kv : [BPC*128, CP*N]  fp8,  kv[b*128+p, dj*N+n] = txt[b, dj*128+p, n]
    #   kvt: [BPC*128, NJ*C]  fp8,  kvt[b*128+p, nj*C+d] = txt[b, d, nj*128+p]
    img_d = nc.dram_tensor("img", [BPC * 128, CP * N], BF16,
                           kind="ExternalInput")
    qt_d = nc.dram_tensor("qt", [BPC * 128, NJ * C], FP8,
                          kind="ExternalInput")
    kv_d = nc.dram_tensor("kv", [BPC * 128, CP * N], FP8,
                          kind="ExternalInput")
    gam_d = nc.dram_tensor("gam", [1], F32, kind="ExternalInput")
    out_d = nc.dram_tensor("out", [BPC * C, N], BF16, kind="ExternalOutput")

    img_f = img_d.ap()
    qt_f = qt_d.ap()
    kv_f = kv_d.ap()
    out_f = out_d.ap()

    with TileContext(nc) as tc:
        with (
            tc.tile_pool(name="const", bufs=1) as const_pool,
            tc.tile_pool(name="kv", bufs=2) as kv_pool,        # [128,16K] fp8
            tc.tile_pool(name="kvt", bufs=2) as kvt_pool,      # [128,16K] fp8
            tc.tile_pool(name="qt", bufs=2) as qt_pool,        # [128,16K] fp8
            tc.tile_pool(name="img", bufs=8) as img_pool,      # [128,4096] bf16
            tc.tile_pool(name="p", bufs=8) as p_pool,          # [128,512] bf16
            tc.tile_pool(name="pt", bufs=2) as pt_pool,        # [128,2048] fp8
            tc.tile_pool(name="ot", bufs=5) as out_pool,       # [128,2048] bf16
            tc.tile_pool(name="stat", bufs=16) as stat_pool,   # [128,1] f32
            tc.tile_pool(name="sps", bufs=2, space="PSUM") as s_pool,
            tc.tile_pool(name="pts", bufs=1, space="PSUM") as pt_ps_pool,
            tc.tile_pool(name="ips", bufs=3, space="PSUM") as info_pool,
            tc.tile_pool(name="tps", bufs=2, space="PSUM") as tp_pool,
        ):
            ident_bf = const_pool.tile([128, 128], BF16, tag="identbf")
            make_identity(nc, ident_bf[:])
            ident_f8 = const_pool.tile([128, 128], FP8, tag="identf8")
            make_identity(nc, ident_f8[:])
            g1 = const_pool.tile([128, 1], F32, tag="g1")
            nc.sync.dma_start(g1[0:1, 0:1],
                              gam_d.ap().rearrange("(a b) -> a b", a=1))
            g_b = const_pool.tile([128, 1], F32, tag="gb")
            nc.gpsimd.partition_broadcast(g_b[:], g1[0:1, 0:1])

            # ---------------- prologue: all loads on SP ----------------
            # kv first: the kvT transpose pipeline gates everything else
            kv_sb, qt_sb, img_t = [], [], []
            for b in range(BPC):
                prows = slice(b * 128, (b + 1) * 128)
                kvt_ = kv_pool.tile([128, CP * N], FP8, tag="kv",
                                    name=f"kv_{b}")
                for h in range(2):
                    nc.sync.dma_start(kvt_[:, h * 2 * N:(h + 1) * 2 * N],
                                      kv_f[prows, h * 2 * N:(h + 1) * 2 * N])
                qtt = qt_pool.tile([128, NJ * C], FP8, tag="qt",
                                   name=f"qt_{b}")
                for h in range(2):
                    nc.sync.dma_start(qtt[:, h * 8192:(h + 1) * 8192],
                                      qt_f[prows, h * 8192:(h + 1) * 8192])
                imgs = [img_pool.tile([128, N], BF16, tag="img",
                                      name=f"img_{b}_{ci}")
                        for ci in range(CP)]
                for ci in range(CP):
                    nc.sync.dma_start(imgs[ci][:],
                                      img_f[prows, ci * N:(ci + 1) * N])
                kv_sb.append(kvt_)
                qt_sb.append(qtt)
                img_t.append(imgs)

            kv3 = [t[:].rearrange("p (d n) -> p d n", n=N) for t in kv_sb]
            qt3 = [t[:].rearrange("p (j c) -> p j c", c=C) for t in qt_sb]
            kvt_sb = [kvt_pool.tile([128, NJ * C], FP8, tag="kvt",
                                    name=f"kvt_{b}") for b in range(BPC)]
            kvt3 = [t[:].rearrange("p (j d) -> p j d", d=C) for t in kvt_sb]
            kvt4 = [t[:].rearrange("p (j k c) -> p j k c", k=CP, c=128)
                    for t in kvt_sb]

            def emit_quad(b, q):
                """kvT nj-pair q: 8 PE fp8 transposes into a stride-2 PSUM
                tile (HW requires element step 2 for fp8 transpose mode),
                then a pack-evac to SBUF; evac engine alternates ACT/DVE."""
                tp = tp_pool.tile([128, 2048], FP8, tag="tp")
                tv = tp[:].rearrange("p (j k c t) -> p j k c t",
                                     j=2, k=CP, t=2)
                for jl in range(2):
                    nj = q * 2 + jl
                    for dj in range(CP):
                        nc.tensor.transpose(
                            tv[:, jl, dj, :, 0],
                            kv3[b][:, dj, nj * 128:(nj + 1) * 128],
                            ident_f8[:])
                eng = nc.scalar if q % 2 == 0 else nc.vector
                if eng is nc.scalar:
                    eng.copy(kvt4[b][:, 2 * q:2 * q + 2, :, :],
                             tv[:, :, :, :, 0])
                else:
                    eng.tensor_copy(kvt4[b][:, 2 * q:2 * q + 2, :, :],
                                    tv[:, :, :, :, 0])

            for q in range(NT):
                emit_quad(0, q)

            for b in range(BPC):
                # ---------- mm1, ci-outer on a 2-deep S ring ----------
                psc = []
                for ci in range(CP):
                    s_ps = s_pool.tile([128, 512], F32, tag="s",
                                       name=f"s_{b}_{ci}")
                    for t in range(NT):
                        nc.tensor.matmul(
                            s_ps[:],
                            qt3[b][:, 2 * t:2 * t + 2,
                                   ci * 128:(ci + 1) * 128],
                            kvt3[b][:, 2 * t:2 * t + 2, :],
                            start=(t == 0), stop=(t == NT - 1),
                            perf_mode=DR)

                    # ---------- softmax (gamma/rowsum folded into P) ------
                    nm = stat_pool.tile([128, 1], F32, tag="nm")
                    nc.vector.reduce_max(nm[:], s_ps[:], axis=AX,
                                         negate=True)
                    praw = p_pool.tile([128, 512], BF16, tag="praw")
                    rs = stat_pool.tile([128, 1], F32, tag="rs")
                    nc.scalar.activation(praw[:], s_ps[:],
                                         mybir.ActivationFunctionType.Exp,
                                         bias=nm[:], scale=1.0,
                                         accum_out=rs[:])
                    rr = stat_pool.tile([128, 1], F32, tag="rr")
                    nc.vector.reciprocal(rr[:], rs[:])
                    rg = stat_pool.tile([128, 1], F32, tag="rg")
                    nc.vector.tensor_mul(rg[:], rr[:], g_b[:])
                    ps = p_pool.tile([128, 512], BF16, tag="ps")
                    nc.vector.tensor_scalar_mul(ps[:], praw[:], rg[:])
                    psc.append(ps)

                # ---------- pT: bf16 PE transposes, DVE evac -> fp8 -------
                pt_sb = pt_pool.tile([128, CP * C], FP8, tag="pt",
                                     name=f"pt_{b}")
                pt3 = pt_sb[:].rearrange("p (d c) -> p d c", c=C)
                for hh in range(2):
                    ptp = pt_ps_pool.tile([128, 1024], BF16, tag="ptp")
                    pv = ptp[:].rearrange("p (l c) -> p l c", c=128)
                    for cl in range(2):
                        ci = hh * 2 + cl
                        for dj in range(CP):
                            nc.tensor.transpose(
                                pv[:, cl * 4 + dj, :],
                                psc[ci][:, dj * 128:(dj + 1) * 128],
                                ident_bf[:])
                    for cl in range(2):
                        ci = hh * 2 + cl
                        nc.vector.tensor_copy(
                            pt3[:, :, ci * 128:(ci + 1) * 128],
                            pv[:, cl * 4:(cl + 1) * 4, :])

                # ---------- mm2 + residual + store ----------
                res_idx = 0
                for ci in range(CP):
                    orow = slice(b * C + ci * 128, b * C + (ci + 1) * 128)
                    for h in range(2):
                        ot = out_pool.tile([128, 2048], BF16, tag="ot",
                                           name=f"ot_{b}_{ci}_{h}")
                        for u in range(4):
                            nb = h * 4 + u
                            cols = slice(nb * 512, (nb + 1) * 512)
                            ip = info_pool.tile([128, 512], F32, tag="i")
                            for t in range(2):
                                nc.tensor.matmul(
                                    ip[:],
                                    pt3[:, 2 * t:2 * t + 2,
                                        ci * 128:(ci + 1) * 128],
                                    kv3[b][:, 2 * t:2 * t + 2,
                                           nb * 512:(nb + 1) * 512],
                                    start=(t == 0), stop=(t == 1),
                                    perf_mode=DR)
                            oc = ot[:, u * 512:(u + 1) * 512]
                            if res_idx % 2 == 0:
                                nc.vector.tensor_add(oc, ip[:],
                                                     img_t[b][ci][:, cols])
                            else:
                                # PE adds the bf16 residual into the open
                                # accumulator; ACT evacuates
                                nc.tensor.matmul(
                                    ip[:], ident_bf[:],
                                    img_t[b][ci][:, cols],
                                    start=False, stop=True,
                                    skip_group_check=True)
                                nc.scalar.copy(oc, ip[:])
                            res_idx += 1
                        nc.sync.dma_start(
                            out_f[orow, h * 2048:(h + 1) * 2048], ot[:])
                        # software pipeline: next batch's kvT quads ride
                        # inside this batch's mm2 stream
                        if b + 1 < BPC:
                            k = ci * 2 + h
                            emit_quad(b + 1, 2 * k)
                            emit_quad(b + 1, 2 * k + 1)

    nc.compile()
    return nc


def _get_nc():
    global _nc_cache
    if _nc_cache is None:
        _nc_cache = _build_nc()
    return _nc_cache


def kernel(img_feat, text_feat, gamma):
    import ml_dtypes
    from concourse.bass_utils import run_bass_kernel_spmd

    nc = _get_nc()
    BF = ml_dtypes.bfloat16
    F8 = ml_dtypes.float8_e4m3

    img = np.asarray(img_feat, dtype=np.float32).reshape(B, C, N)
    txt = np.asarray(text_feat, dtype=np.float32).reshape(B, C, N)
    g = np.ascontiguousarray(np.asarray(gamma), dtype=np.float32).reshape(1)

    def pmajor(x):      # [B, C, N] -> [B*128, CP*N], partition-major rows
        return x.reshape(B, CP, 128, N).transpose(0, 2, 1, 3).reshape(
            B * 128, CP * N)

    def tmajor(x):      # [B, C, N] -> [B*128, NJ*C], [n, c] partition-major
        return x.transpose(0, 2, 1).reshape(B, NJ, 128, C).transpose(
            0, 2, 1, 3).reshape(B * 128, NJ * C)

    img_bf = np.ascontiguousarray(pmajor(img).astype(BF))
    qt_f8 = np.ascontiguousarray(tmajor(img).astype(F8))
    kv_f8 = np.ascontiguousarray(pmajor(txt).astype(F8))

    R = BPC * 128
    in_maps = [
        {
            "img": img_bf[i * R:(i + 1) * R],
            "qt": qt_f8[i * R:(i + 1) * R],
            "kv": kv_f8[i * R:(i + 1) * R],
            "gam": g,
        }
        for i in range(N_CORES)
    ]
    res = run_bass_kernel_spmd(nc, in_maps, core_ids=list(range(N_CORES)))
    out = np.concatenate(
        [np.asarray(res.results[i]["out"]) for i in range(N_CORES)], axis=0)
    return out.astype(np.float32).reshape(B, C, H, W)
